# revision 2
# baseline (speedup 1.0000x reference)
"""GroupQuantLinear: y = x @ dequant(w).T + b on 8 NeuronCores.

Strategy (column-parallel, fp8 DoubleRow PE path):
  The 4-bit weight W = n*s + t (nibble n in 0..15, per-group scale s, bias t)
  is split as W = (n-7.5)*s + (7.5*s + t).
    - Residual Wq = (n-7.5)*s carries only ~14% of W's variance. It is cast
      to fp8 e4m3 with a per-output-column power-of-2 scale A[o] (exactly
      invertible) and multiplied against fp8(x) using DoubleRow matmuls:
      2 K-slices (K=256) per PE instruction at 1 col/cycle -> 2x fp16 rate.
      Combined fp8 quantization error ~3.8% * sqrt(0.14) = ~1.4% rel.
    - The group-affine part (7.5*s+t) is exact: y += xg @ (7.5s+t).T where
      xg[t,g] = sum of x over group g (host, fp32). Done as one K=65 fp16
      matmul per chunk that also folds in the output bias b (ones row).
  Eject: single DVE multiply by 1/A[o] (replicated fp32), fp16 output DMA.
  PE cost per (token-tile, chunk): 16 DR + 1 aug = 17N cycles vs 32N fp16.
  Shards W/outputs along out_features across 8 cores (1376 each).
"""

import os
import sys
from contextlib import ExitStack

import numpy as np

sys.path.insert(0, "/opt/trn_rl_repo")

TOKENS = 8192
IN_F = 4096
OUT_F = 11008
N_CORES = 8
SHARD = OUT_F // N_CORES          # 1376
CHUNKS = (512, 512, 352)          # out-cols per PSUM bank, sum = SHARD
P = 128
KS = IN_F // P                    # 32 k-slices
DR = KS // 2                      # 16 DoubleRow steps
TT = TOKENS // P                  # 64 token tiles
NG = 64                           # quant groups
AUGK = NG + 1                     # 64 group-sum rows + 1 ones row (bias)

_NC_CACHE = {}


def _build_nc():
    import concourse.bacc as bacc
    import concourse.mybir as mybir
    import concourse.tile as tile

    dt8 = mybir.dt.float8e4
    dt16 = mybir.dt.float16
    f32 = mybir.dt.float32
    DRMODE = mybir.MatmulPerfMode.DoubleRow

    nc = bacc.Bacc(
        "TRN2",
        target_bir_lowering=False,
        debug=False,
        enable_asserts=False,
        num_devices=N_CORES,
    )
    xt = nc.dram_tensor("xt", (IN_F, TOKENS), dt8, kind="ExternalInput").ap()
    wt = nc.dram_tensor("wt", (IN_F, SHARD), dt8, kind="ExternalInput").ap()
    xg = nc.dram_tensor("xg", (AUGK, TOKENS), dt16, kind="ExternalInput").ap()
    tpa = nc.dram_tensor("tpa", (AUGK, SHARD), dt16, kind="ExternalInput").ap()
    scl = nc.dram_tensor("scl", (P, SHARD), f32, kind="ExternalInput").ap()
    y = nc.dram_tensor("y", (TOKENS, SHARD), dt16, kind="ExternalOutput").ap()

    coff = [0]
    for ch in CHUNKS:
        coff.append(coff[-1] + ch)

    with tile.TileContext(nc) as tc, ExitStack() as ctx:
        wpool = ctx.enter_context(tc.tile_pool(name="w", bufs=1))
        xpool = ctx.enter_context(tc.tile_pool(name="x", bufs=4))
        opool = ctx.enter_context(tc.tile_pool(name="o", bufs=6))
        pspool = ctx.enter_context(tc.tile_pool(name="ps", bufs=2, space="PSUM"))

        w_sb = wpool.tile([P, KS, SHARD], dt8, name="w_sb")
        tpa_sb = wpool.tile([AUGK, SHARD], dt16, name="tpa_sb")
        scl_sb = wpool.tile([P, SHARD], f32, name="scl_sb")

        xt_r = xt.rearrange("(ks p) m -> p ks m", p=P)
        wt_r = wt.rearrange("(ks p) n -> p ks n", p=P)

        # PE prewarm: dependency-free dummy matmuls on uninitialized SBUF
        # ramp the PE clock while the first DMAs land.
        warm_in = wpool.tile([P, 2, P], dt8, name="warm_in")
        warm_mv = wpool.tile([P, 2, 256], dt8, name="warm_mv")
        nc.any.memzero(warm_in[:])
        nc.any.memzero(warm_mv[:])
        warm_ps = pspool.tile([P, 256], f32, name="warm_ps", tag="warm", bufs=1)
        for _ in range(60):
            nc.tensor.matmul(warm_ps[:], warm_in[:], warm_mv[:],
                             perf_mode=DRMODE, start=True, stop=True)

        # Early loads: x ks-slices land just ahead of consumption by the
        # t0/t1-interleaved loop; W streams in ks-major slabs behind them.
        x0 = xpool.tile([P, KS, P], dt8, name="x_sb", tag="x_sb")
        x1 = xpool.tile([P, KS, P], dt8, name="x_sb", tag="x_sb")
        xg0 = xpool.tile([AUGK, P], dt16, name="xg_sb", tag="xg_sb")
        xg1 = xpool.tile([AUGK, P], dt16, name="xg_sb", tag="xg_sb")
        nc.sync.dma_start(x0[:, 0:4, :], xt_r[:, 0:4, 0:P])
        nc.sync.dma_start(x1[:, 0:4, :], xt_r[:, 0:4, P:2 * P])
        q4 = SHARD // 4
        for q in range(4):
            nc.sync.dma_start(
                w_sb[:, 0:1, q * q4:(q + 1) * q4], wt_r[:, 0:1, q * q4:(q + 1) * q4]
            )
        nc.sync.dma_start(x0[:, 4:16, :], xt_r[:, 4:16, 0:P])
        nc.sync.dma_start(x1[:, 4:16, :], xt_r[:, 4:16, P:2 * P])
        half = SHARD // 2
        for s in range(1, 4):
            nc.sync.dma_start(w_sb[:, s:s + 1, :half], wt_r[:, s:s + 1, :half])
            nc.sync.dma_start(w_sb[:, s:s + 1, half:], wt_r[:, s:s + 1, half:])
        nc.sync.dma_start(x0[:, 16:KS, :], xt_r[:, 16:KS, 0:P])
        nc.sync.dma_start(x1[:, 16:KS, :], xt_r[:, 16:KS, P:2 * P])
        nc.sync.dma_start(xg0[:], xg[:, 0:P])
        nc.sync.dma_start(xg1[:], xg[:, P:2 * P])
        for s in range(4, KS):
            nc.sync.dma_start(w_sb[:, s:s + 1, :], wt_r[:, s:s + 1, :])
        nc.sync.dma_start(tpa_sb[:], tpa)
        nc.sync.dma_start(scl_sb[:], scl)

        def eject(t, c, ps):
            o_sb = opool.tile([P, 512], dt16, name="o_sb", tag="o_sb")[:, :CHUNKS[c]]
            nc.vector.tensor_mul(o_sb[:], ps[:], scl_sb[:, coff[c]:coff[c + 1]])
            nc.sync.dma_start(y[t * P:(t + 1) * P, coff[c]:coff[c + 1]], o_sb[:])

        def tile_matmuls(x_sb, xg_sb, pss):
            for s in range(DR):
                for c in range(len(CHUNKS)):
                    nc.tensor.matmul(
                        pss[c][:],
                        x_sb[:, 2 * s:2 * s + 2, :],
                        w_sb[:, 2 * s:2 * s + 2, coff[c]:coff[c + 1]],
                        perf_mode=DRMODE,
                        start=(s == 0),
                        stop=False,
                    )
            for c in range(len(CHUNKS)):
                nc.tensor.matmul(
                    pss[c][:],
                    xg_sb[:],
                    tpa_sb[:, coff[c]:coff[c + 1]],
                    start=False,
                    stop=True,
                )

        # t = 0 and 1 interleaved over ks so combined compute covers the
        # W-load tail.
        pss01 = [
            [
                pspool.tile([P, CHUNKS[c]], f32, name=f"ps{c}", tag=f"ps{c}")
                for c in range(len(CHUNKS))
            ]
            for _ in range(2)
        ]
        for s in range(DR):
            for tt in range(2):
                x_sb = x0 if tt == 0 else x1
                for c in range(len(CHUNKS)):
                    nc.tensor.matmul(
                        pss01[tt][c][:],
                        x_sb[:, 2 * s:2 * s + 2, :],
                        w_sb[:, 2 * s:2 * s + 2, coff[c]:coff[c + 1]],
                        perf_mode=DRMODE,
                        start=(s == 0),
                        stop=False,
                    )
        for tt in range(2):
            xg_sb = xg0 if tt == 0 else xg1
            for c in range(len(CHUNKS)):
                nc.tensor.matmul(
                    pss01[tt][c][:],
                    xg_sb[:],
                    tpa_sb[:, coff[c]:coff[c + 1]],
                    start=False,
                    stop=True,
                )
        for tt in range(2):
            for c in range(len(CHUNKS)):
                eject(tt, c, pss01[tt][c])

        for t in range(2, TT):
            x_sb = xpool.tile([P, KS, P], dt8, name="x_sb", tag="x_sb")
            xg_sb = xpool.tile([AUGK, P], dt16, name="xg_sb", tag="xg_sb")
            nc.sync.dma_start(x_sb[:], xt_r[:, :, t * P:(t + 1) * P])
            nc.sync.dma_start(xg_sb[:], xg[:, t * P:(t + 1) * P])

            pss = [
                pspool.tile([P, CHUNKS[c]], f32, name=f"ps{c}", tag=f"ps{c}")
                for c in range(len(CHUNKS))
            ]
            tile_matmuls(x_sb, xg_sb, pss)
            for c in range(len(CHUNKS)):
                eject(t, c, pss[c])

    nc.compile()
    return nc


def _host_prep(x, w_packed, w_scale, w_bias, b):
    import ml_dtypes

    fp8 = ml_dtypes.float8_e4m3

    shifts = np.array([12, 8, 4, 0], dtype=np.int32)
    nib = ((w_packed[..., None] >> shifts) & 15).astype(np.float32)
    n_rows, n_groups, n_ids = w_packed.shape
    n = nib.reshape(n_rows, n_groups, n_ids * 4)         # (out, 64, 64)
    Wq = ((n - 7.5) * w_scale).reshape(n_rows, IN_F)     # residual (out, in)
    Tp = (7.5 * w_scale + w_bias)[..., 0]                # (out, 64)

    mx = np.abs(Wq).max(axis=1)
    mx = np.maximum(mx, 1e-30)
    A = np.exp2(np.floor(np.log2(200.0 / mx))).astype(np.float32)   # (out,)
    WT8 = np.ascontiguousarray((Wq * A[:, None]).T.astype(fp8))     # (in, out)
    TpA = Tp * A[:, None]                                # (out, 64)

    xT8 = np.ascontiguousarray(x.T.astype(fp8))          # (in, tokens)
    xg = x.reshape(TOKENS, NG, IN_F // NG).sum(axis=2)   # (tokens, 64) fp32
    xg1 = np.empty((AUGK, TOKENS), dtype=np.float16)
    xg1[:NG] = xg.T.astype(np.float16)
    xg1[NG] = 1.0

    in_maps = []
    for i in range(N_CORES):
        sl = slice(i * SHARD, (i + 1) * SHARD)
        tpa1 = np.empty((AUGK, SHARD), dtype=np.float16)
        tpa1[:NG] = TpA[sl].T.astype(np.float16)
        tpa1[NG] = (b[sl] * A[sl]).astype(np.float16)
        in_maps.append(
            {
                "xt": xT8,
                "wt": np.ascontiguousarray(WT8[:, sl]),
                "xg": xg1,
                "tpa": tpa1,
                "scl": np.ascontiguousarray(
                    np.broadcast_to((1.0 / A[sl]).astype(np.float32), (P, SHARD))
                ),
            }
        )
    return in_maps


def _run(x, w_packed, w_scale, w_bias, b, trace=False):
    from concourse.bass_utils import run_bass_kernel_spmd

    if "nc" not in _NC_CACHE:
        _NC_CACHE["nc"] = _build_nc()
    nc = _NC_CACHE["nc"]
    in_maps = _host_prep(x, w_packed, w_scale, w_bias, b)
    res = run_bass_kernel_spmd(nc, in_maps, list(range(N_CORES)), trace=trace)
    y = np.concatenate([res.results[i]["y"] for i in range(N_CORES)], axis=1)
    return np.ascontiguousarray(y.astype(np.float32)), res


def kernel(x, w_packed, w_scale, w_bias, b):
    x = np.asarray(x)
    w_packed = np.asarray(w_packed)
    w_scale = np.asarray(w_scale)
    w_bias = np.asarray(w_bias)
    b = np.asarray(b)
    y, _ = _run(x, w_packed, w_scale, w_bias, b, trace=False)
    return y


# revision 4
# speedup vs baseline: 1.0205x; 1.0205x over previous
"""GroupQuantLinear: y = x @ dequant(w).T + b on 8 NeuronCores.

Strategy (column-parallel, fp8 DoubleRow PE path):
  The 4-bit weight W = n*s + t (nibble n in 0..15, per-group scale s, bias t)
  is split as W = (n-7.5)*s + (7.5*s + t).
    - Residual Wq = (n-7.5)*s carries only ~14% of W's variance. It is cast
      to fp8 e4m3 with a per-output-column power-of-2 scale A[o] (exactly
      invertible) and multiplied against fp8(x) using DoubleRow matmuls:
      2 K-slices (K=256) per PE instruction at 1 col/cycle -> 2x fp16 rate.
      Combined fp8 quantization error ~3.8% * sqrt(0.14) = ~1.4% rel.
    - The group-affine part y_aug = xg @ (7.5s+t).T + b (xg = per-group sums
      of x) is 1.5% of the FLOPs and exact; computed on host in fp32,
      shipped pre-scaled by A as fp16, and added to PSUM by the DVE during
      eject. Device returns A*y in fp32; host multiplies by 1/A.
  PE cost per (token-tile, chunk): 16 DoubleRow matmuls = 16N cycles vs
  32N for fp16 - the pure-fp8 PE floor (~587us/core).
  Shards W/outputs along out_features across 8 cores (1376 each).
"""

import os
import sys
from contextlib import ExitStack

import numpy as np

sys.path.insert(0, "/opt/trn_rl_repo")

TOKENS = 8192
IN_F = 4096
OUT_F = 11008
N_CORES = 8
SHARD = OUT_F // N_CORES          # 1376
CHUNKS = (512, 512, 352)          # out-cols per PSUM bank, sum = SHARD
P = 128
KS = IN_F // P                    # 32 k-slices
DR = KS // 2                      # 16 DoubleRow steps
TT = TOKENS // P                  # 64 token tiles
NG = 64                           # quant groups

_NC_CACHE = {}


def _build_nc():
    import concourse.bacc as bacc
    import concourse.mybir as mybir
    import concourse.tile as tile

    dt8 = mybir.dt.float8e4
    dt16 = mybir.dt.float16
    f32 = mybir.dt.float32
    DRMODE = mybir.MatmulPerfMode.DoubleRow

    nc = bacc.Bacc(
        "TRN2",
        target_bir_lowering=False,
        debug=False,
        enable_asserts=False,
        num_devices=N_CORES,
    )
    xt = nc.dram_tensor("xt", (IN_F, TOKENS), dt8, kind="ExternalInput").ap()
    wt = nc.dram_tensor("wt", (IN_F, SHARD), dt8, kind="ExternalInput").ap()
    ya = nc.dram_tensor("ya", (TOKENS, SHARD), dt16, kind="ExternalInput").ap()
    y = nc.dram_tensor("y", (TOKENS, SHARD), f32, kind="ExternalOutput").ap()

    coff = [0]
    for ch in CHUNKS:
        coff.append(coff[-1] + ch)

    with tile.TileContext(nc) as tc, ExitStack() as ctx:
        wpool = ctx.enter_context(tc.tile_pool(name="w", bufs=1))
        xpool = ctx.enter_context(tc.tile_pool(name="x", bufs=4))
        opool = ctx.enter_context(tc.tile_pool(name="o", bufs=6))
        pspool = ctx.enter_context(tc.tile_pool(name="ps", bufs=2, space="PSUM"))

        w_sb = wpool.tile([P, KS, SHARD], dt8, name="w_sb")

        xt_r = xt.rearrange("(ks p) m -> p ks m", p=P)
        wt_r = wt.rearrange("(ks p) n -> p ks n", p=P)

        # PE prewarm: dependency-free dummy matmuls on uninitialized SBUF
        # ramp the PE clock while the first DMAs land.
        warm_in = wpool.tile([P, 2, P], dt8, name="warm_in")
        warm_mv = wpool.tile([P, 2, 256], dt8, name="warm_mv")
        nc.any.memzero(warm_in[:])
        nc.any.memzero(warm_mv[:])
        warm_ps = pspool.tile([P, 256], f32, name="warm_ps", tag="warm", bufs=1)
        for _ in range(60):
            nc.tensor.matmul(warm_ps[:], warm_in[:], warm_mv[:],
                             perf_mode=DRMODE, start=True, stop=True)

        # Early loads: x ks-slices land just ahead of consumption by the
        # t0/t1-interleaved loop; W streams in ks-major slabs behind them.
        x0 = xpool.tile([P, KS, P], dt8, name="x_sb", tag="x_sb")
        x1 = xpool.tile([P, KS, P], dt8, name="x_sb", tag="x_sb")
        ya0 = xpool.tile([P, SHARD], dt16, name="ya_sb", tag="ya_sb")
        ya1 = xpool.tile([P, SHARD], dt16, name="ya_sb", tag="ya_sb")
        nc.sync.dma_start(x0[:, 0:4, :], xt_r[:, 0:4, 0:P])
        nc.sync.dma_start(x1[:, 0:4, :], xt_r[:, 0:4, P:2 * P])
        q4 = SHARD // 4
        for s in range(2):
            for q in range(2):
                nc.sync.dma_start(
                    w_sb[:, s:s + 1, q * 2 * q4:(q + 1) * 2 * q4],
                    wt_r[:, s:s + 1, q * 2 * q4:(q + 1) * 2 * q4],
                )
        nc.sync.dma_start(x0[:, 4:16, :], xt_r[:, 4:16, 0:P])
        nc.sync.dma_start(x1[:, 4:16, :], xt_r[:, 4:16, P:2 * P])
        half = SHARD // 2
        for s in range(2, 6):
            nc.sync.dma_start(w_sb[:, s:s + 1, :half], wt_r[:, s:s + 1, :half])
            nc.sync.dma_start(w_sb[:, s:s + 1, half:], wt_r[:, s:s + 1, half:])
        nc.sync.dma_start(x0[:, 16:KS, :], xt_r[:, 16:KS, 0:P])
        nc.sync.dma_start(x1[:, 16:KS, :], xt_r[:, 16:KS, P:2 * P])
        nc.sync.dma_start(ya0[:], ya[0:P, :])
        nc.sync.dma_start(ya1[:], ya[P:2 * P, :])
        for s in range(6, KS):
            nc.sync.dma_start(w_sb[:, s:s + 1, :], wt_r[:, s:s + 1, :])

        def eject(t, c, ps, ya_sb):
            o_sb = opool.tile([P, 512], f32, name="o_sb", tag="o_sb")[:, :CHUNKS[c]]
            nc.vector.tensor_add(o_sb[:], ps[:], ya_sb[:, coff[c]:coff[c + 1]])
            nc.sync.dma_start(y[t * P:(t + 1) * P, coff[c]:coff[c + 1]], o_sb[:])

        # t = 0 and 1 interleaved over ks so combined compute covers the
        # W-load tail.
        pss01 = [
            [
                pspool.tile([P, CHUNKS[c]], f32, name=f"ps{c}", tag=f"ps{c}")
                for c in range(len(CHUNKS))
            ]
            for _ in range(2)
        ]
        for s in range(DR):
            for tt in range(2):
                x_sb = x0 if tt == 0 else x1
                for c in range(len(CHUNKS)):
                    nc.tensor.matmul(
                        pss01[tt][c][:],
                        x_sb[:, 2 * s:2 * s + 2, :],
                        w_sb[:, 2 * s:2 * s + 2, coff[c]:coff[c + 1]],
                        perf_mode=DRMODE,
                        start=(s == 0),
                        stop=(s == DR - 1),
                    )
        for tt in range(2):
            for c in range(len(CHUNKS)):
                eject(tt, c, pss01[tt][c], ya0 if tt == 0 else ya1)

        for t in range(2, TT):
            x_sb = xpool.tile([P, KS, P], dt8, name="x_sb", tag="x_sb")
            ya_sb = xpool.tile([P, SHARD], dt16, name="ya_sb", tag="ya_sb")
            nc.sync.dma_start(x_sb[:], xt_r[:, :, t * P:(t + 1) * P])
            nc.sync.dma_start(ya_sb[:], ya[t * P:(t + 1) * P, :])

            pss = [
                pspool.tile([P, CHUNKS[c]], f32, name=f"ps{c}", tag=f"ps{c}")
                for c in range(len(CHUNKS))
            ]
            for s in range(DR):
                for c in range(len(CHUNKS)):
                    nc.tensor.matmul(
                        pss[c][:],
                        x_sb[:, 2 * s:2 * s + 2, :],
                        w_sb[:, 2 * s:2 * s + 2, coff[c]:coff[c + 1]],
                        perf_mode=DRMODE,
                        start=(s == 0),
                        stop=(s == DR - 1),
                    )
            for c in range(len(CHUNKS)):
                eject(t, c, pss[c], ya_sb)

    nc.compile()
    return nc


def _host_prep(x, w_packed, w_scale, w_bias, b):
    import ml_dtypes

    fp8 = ml_dtypes.float8_e4m3

    shifts = np.array([12, 8, 4, 0], dtype=np.int32)
    nib = ((w_packed[..., None] >> shifts) & 15).astype(np.float32)
    n_rows, n_groups, n_ids = w_packed.shape
    n = nib.reshape(n_rows, n_groups, n_ids * 4)         # (out, 64, 64)
    Wq = ((n - 7.5) * w_scale).reshape(n_rows, IN_F)     # residual (out, in)
    Tp = (7.5 * w_scale + w_bias)[..., 0]                # (out, 64)

    # exact group-affine part, computed in fp32 on host
    xg = x.reshape(TOKENS, NG, IN_F // NG).sum(axis=2)   # (tokens, 64)
    yaug = xg @ Tp.T + b[None, :]                        # (tokens, out)

    mx = np.abs(Wq).max(axis=1)
    mx = np.maximum(mx, 1e-30)
    A = np.exp2(np.floor(np.log2(128.0 / mx))).astype(np.float32)   # (out,)
    # keep the fp16-shipped yaug*A comfortably inside fp16 range
    ymax = np.abs(yaug).max(axis=0)
    bad = (ymax * A) > 50000.0
    while bad.any():
        A[bad] *= 0.5
        bad = (ymax * A) > 50000.0

    WT8 = np.ascontiguousarray((Wq * A[:, None]).T.astype(fp8))  # (in, out)
    xT8 = np.ascontiguousarray(x.T.astype(fp8))          # (in, tokens)
    yaugA = (yaug * A[None, :]).astype(np.float16)       # (tokens, out)

    in_maps = []
    for i in range(N_CORES):
        sl = slice(i * SHARD, (i + 1) * SHARD)
        in_maps.append(
            {
                "xt": xT8,
                "wt": np.ascontiguousarray(WT8[:, sl]),
                "ya": np.ascontiguousarray(yaugA[:, sl]),
            }
        )
    return in_maps, A


def _run(x, w_packed, w_scale, w_bias, b, trace=False):
    from concourse.bass_utils import run_bass_kernel_spmd

    if "nc" not in _NC_CACHE:
        _NC_CACHE["nc"] = _build_nc()
    nc = _NC_CACHE["nc"]
    in_maps, A = _host_prep(x, w_packed, w_scale, w_bias, b)
    res = run_bass_kernel_spmd(nc, in_maps, list(range(N_CORES)), trace=trace)
    y = np.concatenate([res.results[i]["y"] for i in range(N_CORES)], axis=1)
    y *= (1.0 / A)[None, :]
    return np.ascontiguousarray(y.astype(np.float32)), res


def kernel(x, w_packed, w_scale, w_bias, b):
    x = np.asarray(x)
    w_packed = np.asarray(w_packed)
    w_scale = np.asarray(w_scale)
    w_bias = np.asarray(w_bias)
    b = np.asarray(b)
    y, _ = _run(x, w_packed, w_scale, w_bias, b, trace=False)
    return y
